# revision 7
# baseline (speedup 1.0000x reference)
"""Fused attention block (nn_Attention_27865747817251) on 8 trn2 NeuronCores.

Reference math (per batch b):
  y = x @ w_qkv + b_qkv                      # (L, 3D), D=2048, L=2048
  raw reshape (L, 3D) -> (3, NH, L, HD)      # NH=16, HD=128
  => per-head Q/K/V blocks are CONTIGUOUS ranges of y.flatten():
     q_h = flat[(0*NH+h)*L*HD : ...], k_h = flat[(NH+h)*L*HD : ...], v_h = ...
  A = softmax((K_h @ Q_h^T)/sqrt(HD), axis=-1);  out_h = A @ V_h
  out_bld[b, i, h*HD:(h+1)*HD] = out_h[i, :];  final = out_bld @ w_out + b_out

Sharding (8 cores):
  Launch A: qkv projection as y^T chunks, 2D grid: core k = (batch k//4,
    column-group k%4 of 12 chunks each). DMAs are strip-ordered so the PE
    starts after the first strips land instead of after the full input.
  Host: reassembles flat y, slices per-head Q^T/K^T/V.
  Launch B: core k = (batch, head-group g): attention for heads 4g..4g+3
    (S^T formulation; softmax sum via ones-matmul; A@V accumulated in PSUM)
    + row-parallel output projection -> partial (L, D) sums.
  Host: sums the 4 partials per batch, adds b_out.
"""

from contextlib import ExitStack

import numpy as np
import ml_dtypes

import concourse.bass as bass
from concourse import bacc
import concourse.mybir as mybir
import concourse.tile as tile
from concourse.bass_utils import run_bass_kernel_spmd

B, L, D = 2, 2048, 2048
NH, HD = 16, 128
D3 = 3 * D                      # 6144
NCHUNK = D3 // 128              # 48 column chunks of y
CPC = NCHUNK // 4               # 12 chunks per core (launch A, 4 col groups)
KT = D // 128                   # 16 k-subtiles
SCALE = 1.0 / float(np.sqrt(HD))

MM_DT = mybir.dt.bfloat16       # matmul operand dtype
NP_DT = ml_dtypes.bfloat16

_CACHE = {}


def _build_launch_a(reps=1):
    """Core k=(b, cg): y^T chunks [CPC, 128, L] = (w_qkv col-slice)^T @ x_b^T."""
    nc = bacc.Bacc()
    wq = nc.dram_tensor("wq", [128, KT, CPC * 128], MM_DT, kind="ExternalInput")
    xt = nc.dram_tensor("xt", [128, KT, L], MM_DT, kind="ExternalInput")
    yt = nc.dram_tensor("yt", [CPC, 128, L], MM_DT, kind="ExternalOutput")

    with tile.TileContext(nc) as tc, ExitStack() as ctx:
        singles = ctx.enter_context(tc.tile_pool(name="singles", bufs=1))
        outs = ctx.enter_context(tc.tile_pool(name="outs", bufs=4))
        psum = ctx.enter_context(tc.tile_pool(name="psum", bufs=4, space="PSUM"))

        for _rep in range(reps):
            wq_sb = singles.tile([128, KT, CPC * 128], MM_DT, tag="wq")
            xt_sb = singles.tile([128, KT, L], MM_DT, tag="xt")
            # strip-ordered loads: the first output tile only needs strip 0
            # of each, so PE work starts a few us in instead of after 14MB
            NRB = L // 512
            nc.sync.dma_start(xt_sb[:, :, 0:512], xt[:, :, 0:512])
            nc.sync.dma_start(wq_sb[:, :, 0:128], wq[:, :, 0:128])
            for rb in range(1, NRB):
                nc.sync.dma_start(xt_sb[:, :, rb * 512:(rb + 1) * 512],
                                  xt[:, :, rb * 512:(rb + 1) * 512])
            for cb in range(1, CPC):
                nc.sync.dma_start(wq_sb[:, :, cb * 128:(cb + 1) * 128],
                                  wq[:, :, cb * 128:(cb + 1) * 128])
            for cb in range(CPC):
                for rb in range(NRB):
                    pt = psum.tile([128, 512], mybir.dt.float32, tag="p")
                    for kt in range(KT):
                        nc.tensor.matmul(
                            pt[:],
                            wq_sb[:, kt, cb * 128:(cb + 1) * 128],
                            xt_sb[:, kt, rb * 512:(rb + 1) * 512],
                            start=(kt == 0),
                            stop=(kt == KT - 1),
                        )
                    ot = outs.tile([128, 512], MM_DT, tag="o")
                    nc.scalar.copy(ot[:], pt[:])
                    nc.sync.dma_start(yt[cb, :, rb * 512:(rb + 1) * 512], ot[:])
    nc.compile()
    return nc


def _build_launch_b(reps=1):
    """Core (b,g): attention for 4 heads + row-parallel out-proj partial."""
    HPC = 4                     # heads per core
    nc = bacc.Bacc()
    qt = nc.dram_tensor("qt", [128, HPC, L], MM_DT, kind="ExternalInput")
    kt_ = nc.dram_tensor("kt", [128, HPC, L], MM_DT, kind="ExternalInput")
    v = nc.dram_tensor("v", [128, HPC, L // 128, HD], MM_DT, kind="ExternalInput")
    wo = nc.dram_tensor("wo", [128, HPC, D], MM_DT, kind="ExternalInput")
    fp = nc.dram_tensor("fp", [L, D], mybir.dt.float32, kind="ExternalOutput")

    with tile.TileContext(nc) as tc, ExitStack() as ctx:
        singles = ctx.enter_context(tc.tile_pool(name="singles", bufs=1))
        pts = ctx.enter_context(tc.tile_pool(name="pts", bufs=3))
        norm = ctx.enter_context(tc.tile_pool(name="norm", bufs=2))
        fouts = ctx.enter_context(tc.tile_pool(name="fouts", bufs=3))
        psx = ctx.enter_context(tc.tile_pool(name="psx", bufs=2, space="PSUM"))
        pss = ctx.enter_context(tc.tile_pool(name="pss", bufs=2, space="PSUM"))
        pso = ctx.enter_context(tc.tile_pool(name="pso", bufs=2, space="PSUM"))
        psf = ctx.enter_context(tc.tile_pool(name="psf", bufs=2, space="PSUM"))

        for _rep in range(reps):
            qt_sb = singles.tile([128, HPC, L], MM_DT, tag="qt")
            kt_sb = singles.tile([128, HPC, L], MM_DT, tag="kt")
            v_sb = singles.tile([128, HPC, L // 128, HD], MM_DT, tag="v")
            wo_sb = singles.tile([128, HPC, D], MM_DT, tag="wo")
            ones_sb = singles.tile([128, 128], MM_DT, tag="ones")
            outT_sb = singles.tile([128, HPC, L], MM_DT, tag="outT")
            # per-head chunked loads so head 0's attention starts early
            for hh in range(HPC):
                nc.sync.dma_start(qt_sb[:, hh, :], qt[:, hh, :])
                nc.sync.dma_start(kt_sb[:, hh, :], kt_[:, hh, :])
                nc.sync.dma_start(v_sb[:, hh, :, :], v[:, hh, :, :])
            nc.sync.dma_start(wo_sb[:], wo[:])
            nc.vector.memset(ones_sb[:], 1.0)

            NIB = 4             # i blocks of 512 (K rows = output tokens)
            NJB = L // 128      # 16 j blocks (softmax dim)
            for ib in range(NIB):
                for hh in range(HPC):
                    ps_s = pss.tile([128, 512], mybir.dt.float32, tag="s")
                    ps_o = pso.tile([128, 512], mybir.dt.float32, tag="o")
                    for jb in range(NJB):
                        ps_x = psx.tile([128, 512], mybir.dt.float32, tag="x")
                        nc.tensor.matmul(
                            ps_x[:],
                            qt_sb[:, hh, jb * 128:(jb + 1) * 128],
                            kt_sb[:, hh, ib * 512:(ib + 1) * 512],
                            start=True, stop=True,
                        )
                        pT = pts.tile([128, 512], MM_DT, tag="pT")
                        nc.scalar.activation(
                            pT[:], ps_x[:], mybir.ActivationFunctionType.Exp,
                            scale=SCALE,
                        )
                        nc.tensor.matmul(ps_s[:], ones_sb[:], pT[:],
                                         start=(jb == 0), stop=(jb == NJB - 1))
                        nc.tensor.matmul(ps_o[:], v_sb[:, hh, jb, :], pT[:],
                                         start=(jb == 0), stop=(jb == NJB - 1))
                    recip = norm.tile([128, 512], mybir.dt.float32, tag="r")
                    nc.vector.reciprocal(recip[:], ps_s[:])
                    nc.vector.tensor_mul(
                        out=outT_sb[:, hh, ib * 512:(ib + 1) * 512],
                        in0=ps_o[:], in1=recip[:],
                    )
                # out-proj for this i block's 4 token r-blocks (tokens = i)
                for rr in range(4):
                    rb = ib * 4 + rr
                    for cb in range(4):
                        ps_f = psf.tile([128, 512], mybir.dt.float32, tag="f")
                        for hh in range(HPC):
                            nc.tensor.matmul(
                                ps_f[:],
                                outT_sb[:, hh, rb * 128:(rb + 1) * 128],
                                wo_sb[:, hh, cb * 512:(cb + 1) * 512],
                                start=(hh == 0), stop=(hh == 3),
                            )
                        fo = fouts.tile([128, 512], mybir.dt.float32, tag="fo")
                        nc.scalar.copy(fo[:], ps_f[:])
                        nc.sync.dma_start(
                            fp[rb * 128:(rb + 1) * 128,
                               cb * 512:(cb + 1) * 512],
                            fo[:],
                        )
    nc.compile()
    return nc


def _get(name, reps=1):
    key = (name, reps)
    if key not in _CACHE:
        _CACHE[key] = (_build_launch_a(reps) if name == "a"
                       else _build_launch_b(reps))
    return _CACHE[key]


def _prep_a(x, w_qkv):
    in_a = []
    for k in range(8):
        b, cg = k // 4, k % 4
        wsl = w_qkv[:, cg * CPC * 128:(cg + 1) * CPC * 128]
        wq_h = np.ascontiguousarray(
            wsl.reshape(KT, 128, CPC * 128).transpose(1, 0, 2)).astype(NP_DT)
        xt_h = np.ascontiguousarray(
            x[b].T.reshape(KT, 128, L).transpose(1, 0, 2)).astype(NP_DT)
        in_a.append({"wq": wq_h, "xt": xt_h})
    return in_a


def _prep_b(ya_list, b_qkv, w_out):
    """ya_list: 8 arrays [CPC, 128, L]; returns per-core launch-B inputs."""
    sec = L * HD
    in_b = []
    for b in range(B):
        yb = np.concatenate([ya_list[b * 4 + cg] for cg in range(4)], axis=0)
        if b_qkv.any():
            yb = (yb.astype(np.float32)
                  + b_qkv.reshape(NCHUNK, 128)[:, :, None]).astype(NP_DT)
        flat = np.ascontiguousarray(yb.transpose(2, 0, 1)).reshape(-1)
        for g in range(4):
            qts, kts, vs = [], [], []
            for hh in range(4):
                h = 4 * g + hh
                qh = flat[h * sec:(h + 1) * sec].reshape(L, HD)
                kh = flat[(NH + h) * sec:(NH + h + 1) * sec].reshape(L, HD)
                vh = flat[(2 * NH + h) * sec:(2 * NH + h + 1) * sec].reshape(L, HD)
                qts.append(qh.T)
                kts.append(kh.T)
                vs.append(vh.reshape(L // 128, 128, HD).transpose(1, 0, 2))
            wsl = w_out[g * 512:(g + 1) * 512, :]
            wo_h = np.ascontiguousarray(
                wsl.reshape(4, 128, D).transpose(1, 0, 2)).astype(NP_DT)
            in_b.append({
                "qt": np.ascontiguousarray(np.stack(qts, axis=1)),
                "kt": np.ascontiguousarray(np.stack(kts, axis=1)),
                "v": np.ascontiguousarray(np.stack(vs, axis=1)),
                "wo": wo_h,
            })
    return in_b


def kernel(x, w_qkv, b_qkv, w_out, b_out, _timing=None):
    x = np.asarray(x, dtype=np.float32)
    w_qkv = np.asarray(w_qkv, dtype=np.float32)
    b_qkv = np.asarray(b_qkv, dtype=np.float32)
    w_out = np.asarray(w_out, dtype=np.float32)
    b_out = np.asarray(b_out, dtype=np.float32)
    cores = list(range(8))

    in_a = _prep_a(x, w_qkv)
    res_a = run_bass_kernel_spmd(_get("a"), in_a, cores)
    ya = [np.asarray(res_a.results[k]["yt"]) for k in range(8)]

    in_b = _prep_b(ya, b_qkv, w_out)
    res_b = run_bass_kernel_spmd(_get("b"), in_b, cores)

    out = np.empty((B, L, D), dtype=np.float32)
    for b in range(B):
        acc = np.zeros((L, D), dtype=np.float32)
        for g in range(4):
            acc += np.asarray(res_b.results[b * 4 + g]["fp"])
        out[b] = acc + b_out[None, :]
    return out


# revision 17
# speedup vs baseline: 1.1027x; 1.1027x over previous
"""Fused attention block (nn_Attention_27865747817251) on 8 trn2 NeuronCores.

Reference math (per batch b):
  y = x @ w_qkv + b_qkv                      # (L, 3D), D=2048, L=2048
  raw reshape (L, 3D) -> (3, NH, L, HD)      # NH=16, HD=128
  => per-head Q/K/V blocks are CONTIGUOUS ranges of y.flatten():
     q_h = flat[(0*NH+h)*L*HD : ...], k_h = flat[(NH+h)*L*HD : ...], v_h = ...
  A = softmax((K_h @ Q_h^T)/sqrt(HD), axis=-1);  out_h = A @ V_h
  out_bld[b, i, h*HD:(h+1)*HD] = out_h[i, :];  final = out_bld @ w_out + b_out

Sharding (8 cores):
  Launch A: qkv projection as y^T chunks, 2D grid: core k = (batch k//4,
    column-group k%4 of 12 chunks each). DMAs are strip-ordered so the PE
    starts after the first strips land instead of after the full input.
  Host: reassembles flat y, slices per-head Q^T/K^T/V.
  Launch B: core k = (batch, head-group g): attention for heads 4g..4g+3
    (S^T formulation; softmax sum via ones-matmul; A@V accumulated in PSUM)
    + row-parallel output projection -> partial (L, D) sums.
  Host: sums the 4 partials per batch, adds b_out.
"""

from contextlib import ExitStack

import numpy as np
import ml_dtypes

import concourse.bass as bass
from concourse import bacc
import concourse.mybir as mybir
import concourse.tile as tile
from concourse.bass_utils import run_bass_kernel_spmd

B, L, D = 2, 2048, 2048
NH, HD = 16, 128
D3 = 3 * D                      # 6144
NCHUNK = D3 // 128              # 48 column chunks of y
CPC = NCHUNK // 4               # 12 chunks per core (launch A, 4 col groups)
KT = D // 128                   # 16 k-subtiles
SCALE = 1.0 / float(np.sqrt(HD))

MM_DT = mybir.dt.bfloat16       # matmul operand dtype
NP_DT = ml_dtypes.bfloat16

_CACHE = {}


def _build_launch_a(reps=1):
    """Core k=(b, cg): y^T chunks [CPC, 128, L] = (w_qkv col-slice)^T @ x_b^T."""
    nc = bacc.Bacc()
    wq = nc.dram_tensor("wq", [128, KT, CPC * 128], MM_DT, kind="ExternalInput")
    xt = nc.dram_tensor("xt", [128, KT, L], MM_DT, kind="ExternalInput")
    yt = nc.dram_tensor("yt", [CPC, 128, L], MM_DT, kind="ExternalOutput")

    with tile.TileContext(nc) as tc, ExitStack() as ctx:
        singles = ctx.enter_context(tc.tile_pool(name="singles", bufs=1))
        outs = ctx.enter_context(tc.tile_pool(name="outs", bufs=4))
        psum = ctx.enter_context(tc.tile_pool(name="psum", bufs=4, space="PSUM"))

        for _rep in range(reps):
            wq_sb = singles.tile([128, KT, CPC * 128], MM_DT, tag="wq")
            xt_sb = singles.tile([128, KT, L], MM_DT, tag="xt")
            # strip-ordered loads: the first output tile only needs strip 0
            # of each, so PE work starts a few us in instead of after 14MB
            NRB = L // 512
            nc.sync.dma_start(xt_sb[:, :, 0:512], xt[:, :, 0:512])
            nc.sync.dma_start(wq_sb[:, :, 0:128], wq[:, :, 0:128])
            for rb in range(1, NRB):
                nc.sync.dma_start(xt_sb[:, :, rb * 512:(rb + 1) * 512],
                                  xt[:, :, rb * 512:(rb + 1) * 512])
            for cb in range(1, CPC):
                nc.sync.dma_start(wq_sb[:, :, cb * 128:(cb + 1) * 128],
                                  wq[:, :, cb * 128:(cb + 1) * 128])
            for cb in range(CPC):
                for rb in range(NRB):
                    pt = psum.tile([128, 512], mybir.dt.float32, tag="p")
                    for kt in range(KT):
                        nc.tensor.matmul(
                            pt[:],
                            wq_sb[:, kt, cb * 128:(cb + 1) * 128],
                            xt_sb[:, kt, rb * 512:(rb + 1) * 512],
                            start=(kt == 0),
                            stop=(kt == KT - 1),
                        )
                    ot = outs.tile([128, 512], MM_DT, tag="o")
                    nc.scalar.copy(ot[:], pt[:])
                    nc.sync.dma_start(yt[cb, :, rb * 512:(rb + 1) * 512], ot[:])
    nc.compile()
    return nc


def _build_launch_b(reps=1):
    """Core (b,g): attention for 4 heads + row-parallel out-proj partial."""
    HPC = 4                     # heads per core
    nc = bacc.Bacc()
    qt = nc.dram_tensor("qt", [128, HPC, L], MM_DT, kind="ExternalInput")
    kt_ = nc.dram_tensor("kt", [128, HPC, L], MM_DT, kind="ExternalInput")
    v = nc.dram_tensor("v", [128, HPC, L // 128, HD], MM_DT, kind="ExternalInput")
    wo = nc.dram_tensor("wo", [128, HPC, D], MM_DT, kind="ExternalInput")
    fp = nc.dram_tensor("fp", [L, D], mybir.dt.float32, kind="ExternalOutput")

    with tile.TileContext(nc) as tc, ExitStack() as ctx:
        singles = ctx.enter_context(tc.tile_pool(name="singles", bufs=1))
        pts = ctx.enter_context(tc.tile_pool(name="pts", bufs=6))
        norm = ctx.enter_context(tc.tile_pool(name="norm", bufs=2))
        fouts = ctx.enter_context(tc.tile_pool(name="fouts", bufs=6))

        for _rep in range(reps):
            qt_sb = singles.tile([128, HPC, L], MM_DT, tag="qt")
            kt_sb = singles.tile([128, HPC, L], MM_DT, tag="kt")
            v_sb = singles.tile([128, HPC, L // 128, HD], MM_DT, tag="v")
            wo_sb = singles.tile([128, HPC, D], MM_DT, tag="wo")
            ones_sb = singles.tile([128, 128], MM_DT, tag="ones")
            outT_sb = singles.tile([128, HPC, L], MM_DT, tag="outT")
            # per-head chunked loads so head 0's attention starts early
            for hh in range(HPC):
                nc.sync.dma_start(qt_sb[:, hh, :], qt[:, hh, :])
                nc.sync.dma_start(kt_sb[:, hh, :], kt_[:, hh, :])
                nc.sync.dma_start(v_sb[:, hh, :, :], v[:, hh, :, :])
            nc.sync.dma_start(wo_sb[:], wo[:])
            nc.vector.memset(ones_sb[:], 1.0)

            NIB = 4             # i blocks of 512 (K rows = output tokens)
            NJB = L // 128      # 16 j blocks (softmax dim)
            with ExitStack() as attn_ctx:
                psx = attn_ctx.enter_context(
                    tc.tile_pool(name="psx", bufs=4, space="PSUM"))
                pss = attn_ctx.enter_context(
                    tc.tile_pool(name="pss", bufs=2, space="PSUM"))
                pso = attn_ctx.enter_context(
                    tc.tile_pool(name="pso", bufs=2, space="PSUM"))
                for ib in range(NIB):
                    for hh in range(HPC):
                        ps_s = pss.tile([128, 512], mybir.dt.float32, tag="s")
                        ps_o = pso.tile([128, 512], mybir.dt.float32, tag="o")
                        for jb in range(NJB):
                            ps_x = psx.tile([128, 512], mybir.dt.float32,
                                            tag="x")
                            nc.tensor.matmul(
                                ps_x[:],
                                qt_sb[:, hh, jb * 128:(jb + 1) * 128],
                                kt_sb[:, hh, ib * 512:(ib + 1) * 512],
                                start=True, stop=True,
                            )
                            pT = pts.tile([128, 512], MM_DT, tag="pT")
                            nc.scalar.activation(
                                pT[:], ps_x[:],
                                mybir.ActivationFunctionType.Exp,
                                scale=SCALE,
                            )
                            nc.tensor.matmul(
                                ps_s[:], ones_sb[:], pT[:],
                                start=(jb == 0), stop=(jb == NJB - 1))
                            nc.tensor.matmul(
                                ps_o[:], v_sb[:, hh, jb, :], pT[:],
                                start=(jb == 0), stop=(jb == NJB - 1))
                        recip = norm.tile([128, 512], mybir.dt.float32,
                                          tag="r")
                        nc.vector.reciprocal(recip[:], ps_s[:])
                        nc.vector.tensor_mul(
                            out=outT_sb[:, hh, ib * 512:(ib + 1) * 512],
                            in0=ps_o[:], in1=recip[:],
                        )
            # out-proj phase (separate PSUM scope — banks reused)
            with tc.tile_pool(name="psf", bufs=6, space="PSUM") as psf:
                for rb in range(L // 128):
                    for cb in range(4):
                        ps_f = psf.tile([128, 512], mybir.dt.float32, tag="f")
                        for hh in range(HPC):
                            nc.tensor.matmul(
                                ps_f[:],
                                outT_sb[:, hh, rb * 128:(rb + 1) * 128],
                                wo_sb[:, hh, cb * 512:(cb + 1) * 512],
                                start=(hh == 0), stop=(hh == 3),
                            )
                        fo = fouts.tile([128, 512], mybir.dt.float32,
                                        tag="fo")
                        if cb % 2 == 0:
                            nc.vector.tensor_copy(fo[:], ps_f[:])
                        else:
                            nc.scalar.copy(fo[:], ps_f[:])
                        nc.sync.dma_start(
                            fp[rb * 128:(rb + 1) * 128,
                               cb * 512:(cb + 1) * 512],
                            fo[:],
                        )
    nc.compile()
    return nc


def _get(name, reps=1):
    key = (name, reps)
    if key not in _CACHE:
        _CACHE[key] = (_build_launch_a(reps) if name == "a"
                       else _build_launch_b(reps))
    return _CACHE[key]


def _prep_a(x, w_qkv):
    in_a = []
    for k in range(8):
        b, cg = k // 4, k % 4
        wsl = w_qkv[:, cg * CPC * 128:(cg + 1) * CPC * 128]
        wq_h = np.ascontiguousarray(
            wsl.reshape(KT, 128, CPC * 128).transpose(1, 0, 2)).astype(NP_DT)
        xt_h = np.ascontiguousarray(
            x[b].T.reshape(KT, 128, L).transpose(1, 0, 2)).astype(NP_DT)
        in_a.append({"wq": wq_h, "xt": xt_h})
    return in_a


def _prep_b(ya_list, b_qkv, w_out):
    """ya_list: 8 arrays [CPC, 128, L]; returns per-core launch-B inputs."""
    sec = L * HD
    in_b = []
    for b in range(B):
        yb = np.concatenate([ya_list[b * 4 + cg] for cg in range(4)], axis=0)
        if b_qkv.any():
            yb = (yb.astype(np.float32)
                  + b_qkv.reshape(NCHUNK, 128)[:, :, None]).astype(NP_DT)
        flat = np.ascontiguousarray(yb.transpose(2, 0, 1)).reshape(-1)
        for g in range(4):
            qts, kts, vs = [], [], []
            for hh in range(4):
                h = 4 * g + hh
                qh = flat[h * sec:(h + 1) * sec].reshape(L, HD)
                kh = flat[(NH + h) * sec:(NH + h + 1) * sec].reshape(L, HD)
                vh = flat[(2 * NH + h) * sec:(2 * NH + h + 1) * sec].reshape(L, HD)
                qts.append(qh.T)
                kts.append(kh.T)
                vs.append(vh.reshape(L // 128, 128, HD).transpose(1, 0, 2))
            wsl = w_out[g * 512:(g + 1) * 512, :]
            wo_h = np.ascontiguousarray(
                wsl.reshape(4, 128, D).transpose(1, 0, 2)).astype(NP_DT)
            in_b.append({
                "qt": np.ascontiguousarray(np.stack(qts, axis=1)),
                "kt": np.ascontiguousarray(np.stack(kts, axis=1)),
                "v": np.ascontiguousarray(np.stack(vs, axis=1)),
                "wo": wo_h,
            })
    return in_b


def kernel(x, w_qkv, b_qkv, w_out, b_out, _timing=None):
    x = np.asarray(x, dtype=np.float32)
    w_qkv = np.asarray(w_qkv, dtype=np.float32)
    b_qkv = np.asarray(b_qkv, dtype=np.float32)
    w_out = np.asarray(w_out, dtype=np.float32)
    b_out = np.asarray(b_out, dtype=np.float32)
    cores = list(range(8))

    in_a = _prep_a(x, w_qkv)
    res_a = run_bass_kernel_spmd(_get("a"), in_a, cores)
    ya = [np.asarray(res_a.results[k]["yt"]) for k in range(8)]

    in_b = _prep_b(ya, b_qkv, w_out)
    res_b = run_bass_kernel_spmd(_get("b"), in_b, cores)

    out = np.empty((B, L, D), dtype=np.float32)
    for b in range(B):
        acc = np.zeros((L, D), dtype=np.float32)
        for g in range(4):
            acc += np.asarray(res_b.results[b * 4 + g]["fp"])
        out[b] = acc + b_out[None, :]
    return out
